# revision 11
# baseline (speedup 1.0000x reference)
"""Multi-head cross-attention TRN2 kernel, v2.

N=4096, D=256, H=4, K=16. Data-parallel over 8 NeuronCores: each core owns
R=512 query rows; key_value + weights replicated. No collectives.

Differences vs v1 baseline (173us/102us):
 - exp split between ACT (hw exp) and DVE (Schraudolph: fp32 bits of
   2^u ~ uint32(A*s + B), one tensor_scalar per tile) -- the exp of the
   8.4M-element score matrix was the single-engine bottleneck.
 - attention matmuls run in float32r (1 cy/row at free>=256): kht/qt/v/es
   stay fp32, PSUM->SBUF moves are plain copies, no bf16 conversion pass.
 - S matmuls for the two heads of a pair issued at row groups {0,32}
   (concurrent on HW); AV matmuls write heads_psum rows 0:32 / 32:64.
 - denominators via ones-column of V_aug (PE accumulates); normalization
   deferred past W_o: per-head o partials scaled by 1/denom with
   per-partition-scalar fused ops (denom transposed to [q,1] via PE).
 - es = exp(0.25*s - C0) with C0=3.466 (uniform shift, softmax-invariant).

Engine budget per core (lane-cycles): exp 65.5K + copies ~20K split across
ACT+DVE (~48us); PE ~83K cy (~35us with packing).
"""
import numpy as np
import ml_dtypes

import concourse.bass as bass
from concourse import bacc
import concourse.mybir as mybir
import concourse.tile as tile
from concourse.bass_utils import run_bass_kernel_spmd

N, D, H, K = 4096, 256, 4, 16
NCORES = 8
R = N // NCORES          # 512 query rows per core
G = K + 1                # 17 cols per head group in v blob (ones + 16 V)
GP = 32                  # padded per-head group stride in v_aug
F32 = mybir.dt.float32
F32R = mybir.dt.float32r
BF16 = mybir.dt.bfloat16
U16 = mybir.dt.uint16
BF = ml_dtypes.bfloat16
EXPF = mybir.ActivationFunctionType.Exp
ALU = mybir.AluOpType

C0 = 3.466               # exponent downshift: es = exp(0.25*s - C0)
LOG2E = float(np.log2(np.e))
# Schraudolph fp32-bit constants: bits = A*s + B  (trunc) viewed as fp32
SCH_A = 0.25 * LOG2E * 128.0
SCH_B = (127.0 - C0 * LOG2E - 0.0547) * 128.0

NCH = N // 128           # 32 key chunks
# exp engine assignment per (pair, chunk): 'A' = ACT, 'D' = DVE.
# ACT gets EXP_ACT of each 32; interleaved.
EXP_ACT = 22


def _exp_engine(m):
    return 'A' if (m * EXP_ACT) % NCH < EXP_ACT else 'D'


TRACE = False
LAST_RESULTS = None


def _build(repeats=1):
    nc = bacc.Bacc()
    qt_d = nc.declare_dram_parameter("qt", [D, R], BF16, isOutput=False)
    kvt_d = nc.declare_dram_parameter("kvt", [D, N], BF16, isOutput=False)
    # wqkv blob [128, 656]: per D-half c at 328c: [wq 0:128 | wk 128:256 | wv 256:324 | pad 324:328]
    # wq/wk head h at cols 32h..32h+16; wv head h at cols 17h..17h+17 (col 17h zero)
    wqkv_d = nc.declare_dram_parameter("wqkv", [128, 656], BF16, isOutput=False)
    # wo blob [64, 512] fp32: pair p cols 256p; head-even rows 1:17, head-odd rows 33:49
    wo_d = nc.declare_dram_parameter("wo", [17, 1024], BF16, isOutput=False)
    ident_d = nc.declare_dram_parameter("ident2", [2, 2], F32, isOutput=False)
    out_d = nc.declare_dram_parameter("out", [R, D], F32, isOutput=True)

    with tile.TileContext(nc) as tc:
        with (
            tc.tile_pool(name="consts", bufs=1) as consts,
            tc.tile_pool(name="es", bufs=4) as espool,
            tc.tile_pool(name="sbw", bufs=2) as sbw,
            tc.tile_pool(name="spsum", bufs=2, space="PSUM") as spsum,
            tc.tile_pool(name="hpsum", bufs=1, space="PSUM") as hpsum,
            tc.tile_pool(name="wpsum", bufs=1, space="PSUM") as wpsum,
            tc.tile_pool(name="vpsum", bufs=1, space="PSUM") as vpsum,
        ):
            for _rep in range(repeats):
                # ---- load weights + transposed activations (host pre-transposed) ----
                wqkv_sb = consts.tile([128, 656], BF16, tag="wqkv_sb", name="wqkv_sb")
                nc.sync.dma_start(out=wqkv_sb, in_=wqkv_d[:, :])
                wo_sb = consts.tile([17, 1024], BF16, tag="wo_sb", name="wo_sb")
                nc.sync.dma_start(out=wo_sb, in_=wo_d[:, :])
                ident2 = consts.tile([2, 2], F32, tag="ident2", name="ident2")
                nc.sync.dma_start(out=ident2, in_=ident_d[:, :])
                biasC0 = consts.tile([128, 1], F32, tag="biasC0", name="biasC0")
                nc.gpsimd.memset(biasC0[:], -C0)

                qt0 = consts.tile([128, R], BF16, tag="qt0", name="qt0")
                qt1 = consts.tile([128, R], BF16, tag="qt1", name="qt1")
                nc.sync.dma_start(out=qt0, in_=qt_d[0:128, :])
                nc.sync.dma_start(out=qt1, in_=qt_d[128:256, :])
                kt0 = consts.tile([128, N], BF16, tag="kt0", name="kt0")
                kt1 = consts.tile([128, N], BF16, tag="kt1", name="kt1")
                for j in range(N // 512):
                    sl = slice(512 * j, 512 * (j + 1))
                    nc.sync.dma_start(out=kt0[:, sl], in_=kvt_d[0:128, sl])
                    nc.sync.dma_start(out=kt1[:, sl], in_=kvt_d[128:256, sl])

                # ---- Q projection: psum [128,512] -> qtA/qtB fp32 ----
                qt_ps = wpsum.tile([128, R], F32, tag="w", name="qt_ps")
                nc.tensor.matmul(qt_ps[:], wqkv_sb[:, 0:128], qt0[:], start=True, stop=False)
                nc.tensor.matmul(qt_ps[:], wqkv_sb[:, 328:456], qt1[:], start=False, stop=True)
                qtA = consts.tile([64, R], BF16, tag="qtA", name="qtA")
                qtB = consts.tile([64, R], BF16, tag="qtB", name="qtB")
                nc.vector.tensor_copy(qtA[:], qt_ps[0:64, :])
                nc.vector.tensor_copy(qtB[:], qt_ps[64:128, :])

                # ---- K/V projections per 512-key chunk ----
                khtA = consts.tile([64, N], BF16, tag="khtA", name="khtA")
                khtB = consts.tile([64, N], BF16, tag="khtB", name="khtB")
                # v_aug: 32 chunks x 4 heads x GP cols; col 0 ones, 1:17 V, 17:32 zero
                v_aug = consts.tile([128, NCH * H * GP], BF16, tag="v_aug", name="v_aug")
                va4 = v_aug[:].rearrange("p (m h s) -> p m h s", h=H, s=GP)
                nc.vector.memset(va4[:, :, :, 0:1], 1.0)
                nc.vector.memset(va4[:, :, :, G:GP], 0.0)
                for j in range(N // 512):
                    sl = slice(512 * j, 512 * (j + 1))
                    kh_ps = wpsum.tile([128, 512], F32, tag="w", name="kh_ps")
                    nc.tensor.matmul(kh_ps[:], wqkv_sb[:, 128:256], kt0[:, sl],
                                     start=True, stop=False)
                    nc.tensor.matmul(kh_ps[:], wqkv_sb[:, 456:584], kt1[:, sl],
                                     start=False, stop=True)
                    # ACT takes the unshifted copy; DVE the partition-shifted one
                    nc.scalar.copy(khtA[:, sl], kh_ps[0:64, :])
                    nc.vector.tensor_copy(khtB[:, sl], kh_ps[64:128, :])
                    v_ps = vpsum.tile([128, 4 * H * G], F32, tag="v", name="v_ps")
                    for i in range(4):
                        isl = slice(128 * (4 * j + i), 128 * (4 * j + i + 1))
                        osl = slice(H * G * i, H * G * (i + 1))
                        nc.tensor.matmul(v_ps[:, osl], kt0[:, isl], wqkv_sb[:, 256:324],
                                         start=True, stop=False)
                        nc.tensor.matmul(v_ps[:, osl], kt1[:, isl], wqkv_sb[:, 584:652],
                                         start=False, stop=True)
                    # one strided copy: [128, 4i, 4h, 16] -> v_aug (skip ones col)
                    vsrc = v_ps[:].rearrange("p (i h s) -> p i h s", h=H, s=G)[:, :, :, 1:G]
                    vdst = va4[:, 4 * j:4 * j + 4, :, 1:G]
                    nc.vector.tensor_copy(vdst, vsrc)

                # ---- attention: head pairs (0,1) on A tiles, (2,3) on B ----
                o_sb = sbw.tile([128, 4 * D], F32, tag="o_sb", name="o_sb", bufs=2)
                for p, (kht_t, qt_t) in enumerate(((khtA, qtA), (khtB, qtB))):
                    heads_e = hpsum.tile([32, R], F32, tag="heads_e", name="heads_e")
                    heads_o = hpsum.tile([64, R], F32, tag="heads_o", name="heads_o")
                    es_tiles = {}

                    def s_stage(m, kht_t=kht_t, qt_t=qt_t, es_tiles=es_tiles, p=p):
                        s_ps = spsum.tile([128, 1024], F32, tag="s", name="s_ps")
                        ksl = slice(128 * m, 128 * (m + 1))
                        nc.tensor.matmul(s_ps[:, 0:512],
                                         kht_t[0:16, ksl],
                                         qt_t[0:16, :],
                                         start=True, stop=True)
                        nc.tensor.matmul(s_ps[:, 512:1024],
                                         kht_t[32:48, ksl],
                                         qt_t[32:48, :],
                                         start=True, stop=True)
                        es = espool.tile([128, 1024], BF16, tag="es", name="es")
                        if _exp_engine(m) == 'A':
                            nc.scalar.activation(es[:], s_ps[:], EXPF,
                                                 bias=biasC0[:], scale=0.25)
                        else:
                            nc.vector.tensor_scalar(
                                es[:].bitcast(U16), s_ps[:], SCH_A, SCH_B,
                                ALU.mult, ALU.add)
                        es_tiles[m] = es

                    def av_stage(m, heads_e=heads_e, heads_o=heads_o, p=p,
                                 es_tiles=es_tiles):
                        es = es_tiles.pop(m)
                        he, ho = 2 * p, 2 * p + 1
                        nc.tensor.matmul(heads_e[0:32, :],
                                         va4[:, m, he, :],
                                         es[:, 0:512],
                                         start=(m == 0), stop=(m == NCH - 1))
                        nc.tensor.matmul(heads_o[32:64, :],
                                         va4[:, m, ho, :],
                                         es[:, 512:1024],
                                         start=(m == 0), stop=(m == NCH - 1))

                    for m in range(NCH + 1):
                        if m < NCH:
                            s_stage(m)
                        if m >= 1:
                            av_stage(m - 1)

                    # ---- normalize + W_o (all operands base-0: 17-row
                    # operands at base 32 hard-fault the PE) ----
                    headsT = sbw.tile([17, 2 * R], BF16, tag="headsT", name="headsT")
                    nc.scalar.copy(headsT[:, 0:R], heads_e[0:17, :])
                    nc.scalar.copy(headsT[:, R:2 * R], heads_o[32:49, :])
                    den_e = sbw.tile([1, R], F32, tag="den_e", name="den_e")
                    den_o = sbw.tile([1, R], F32, tag="den_o", name="den_o")
                    nc.vector.tensor_copy(den_e[:], heads_e[0:1, :])
                    nc.vector.tensor_copy(den_o[:], heads_o[32:33, :])
                    denT = spsum.tile([128, 8], F32, tag="s", name="denT")
                    for c in range(4):
                        csl = slice(128 * c, 128 * (c + 1))
                        nc.tensor.transpose(denT[:, 2 * c:2 * c + 1],
                                            den_e[:, csl], ident2[0:1, 0:1])
                        nc.tensor.transpose(denT[:, 2 * c + 1:2 * c + 2],
                                            den_o[:, csl], ident2[0:1, 0:1])
                    recipT = sbw.tile([128, 8], F32, tag="recipT", name="recipT")
                    nc.vector.reciprocal(recipT[:], denT[:])

                    for c in range(4):
                        csl = slice(128 * c, 128 * (c + 1))
                        o_ps = wpsum.tile([128, 512], F32, tag="w", name="o_ps")
                        nc.tensor.matmul(o_ps[:, 0:256],
                                         headsT[:, csl],
                                         wo_sb[:, 512 * p:512 * p + 256],
                                         start=True, stop=True)
                        nc.tensor.matmul(o_ps[:, 256:512],
                                         headsT[:, R + 128 * c:R + 128 * (c + 1)],
                                         wo_sb[:, 512 * p + 256:512 * (p + 1)],
                                         start=True, stop=True)
                        osl = o_sb[:, D * c:D * (c + 1)]
                        re = recipT[:, 2 * c:2 * c + 1]
                        ro = recipT[:, 2 * c + 1:2 * c + 2]
                        if p == 0:
                            t = sbw.tile([128, D], F32, tag="t", name="t")
                            nc.vector.tensor_scalar(
                                t[:], o_ps[:, 0:256], re, None, ALU.mult)
                            nc.vector.scalar_tensor_tensor(
                                osl, o_ps[:, 256:512], ro, t[:],
                                ALU.mult, ALU.add)
                        else:
                            t = sbw.tile([128, D], F32, tag="t", name="t")
                            nc.vector.scalar_tensor_tensor(
                                t[:], o_ps[:, 0:256], re, osl,
                                ALU.mult, ALU.add)
                            nc.vector.scalar_tensor_tensor(
                                osl, o_ps[:, 256:512], ro, t[:],
                                ALU.mult, ALU.add)
                            nc.sync.dma_start(out=out_d[csl, :], in_=osl)

    nc.finalize()
    return nc


_NC_CACHE = None


def _host_in_maps(query, key_value, W_q, W_k, W_v, W_o):
    qt_full = np.ascontiguousarray(query.astype(BF).T)          # [D, N]
    kvt = np.ascontiguousarray(key_value.astype(BF).T)          # [D, N]
    wqkv_h = np.zeros((128, 656), dtype=BF)
    wqt = np.transpose(W_q, (1, 0, 2))  # [D, H, K]
    wkt = np.transpose(W_k, (1, 0, 2))
    wvt = np.transpose(W_v, (1, 0, 2))
    for half in range(2):
        ds = slice(128 * half, 128 * (half + 1))
        base = 328 * half
        for h in range(H):
            wqkv_h[:, base + 32 * h:base + 32 * h + K] = wqt[ds, h, :].astype(BF)
            wqkv_h[:, base + 128 + 32 * h:base + 128 + 32 * h + K] = \
                wkt[ds, h, :].astype(BF)
            wqkv_h[:, base + 256 + G * h + 1:base + 256 + G * (h + 1)] = \
                wvt[ds, h, :].astype(BF)
    wo_h = np.zeros((17, 1024), dtype=BF)
    wo_r = W_o.reshape(H, K, D)
    for h in range(H):
        wo_h[1:17, 256 * h:256 * (h + 1)] = wo_r[h].astype(BF)
    ident2 = np.eye(2, dtype=np.float32)
    return [{"qt": np.ascontiguousarray(qt_full[:, c * R:(c + 1) * R]),
             "kvt": kvt, "wqkv": wqkv_h, "wo": wo_h, "ident2": ident2}
            for c in range(NCORES)]


def kernel(query, key_value, W_q, W_k, W_v, W_o):
    global _NC_CACHE, LAST_RESULTS
    if _NC_CACHE is None:
        _NC_CACHE = _build()
    nc = _NC_CACHE
    in_maps = _host_in_maps(query, key_value, W_q, W_k, W_v, W_o)
    res = run_bass_kernel_spmd(nc, in_maps, list(range(NCORES)), trace=TRACE)
    LAST_RESULTS = res
    return np.concatenate([res.results[c]["out"] for c in range(NCORES)], axis=0)
